# revision 7
# baseline (speedup 1.0000x reference)
"""Trainium2 Bass kernel for nn_Attention: cross-attention with projections.

Per-core (batch-sharded, 1 of B=8 rows per NeuronCore):
  K = E @ Wk + bk ; V = E @ Wv + bv ; Q = X @ Wq + bq
  S = Q K^T / sqrt(H) ; P = exp(S + mask_add)   (no max-subtraction: S ~ N(0,1))
  O = P @ V / rowsum(P)        (bv inside V is exactly equivalent to adding
                                bv after normalization since weights sum to 1)

Key algebraic folds (all host-side transforms act only on weights/layout):
  * Q K^T = X_aug (Wq_aug Wk_aug^T) E_aug^T, with X_aug=[X|1], so the host
    precomputes MT = Wk_aug @ Wq_aug^T once and the Q projection disappears:
    on-chip Ktilde^T = MT.T-contraction with E_aug^T, then S^T = Ktilde^T
    tiles contracted with X_aug^T. Biases bq/bk ride along exactly.
  * bv is folded into Wv_aug's 1025th row (against the host-appended ones row
    of E_aug), exact because softmax weights sum to one after normalization.
  * All augmented dims are zero-padded to 1152 = 9*128 for uniform tiles.
  * Everything transposed host-side => no on-chip transposes at all.
  * S^T layout [k, q]: softmax denominator = ones-row matmuls over P^T tiles,
    relaid out [1,512] -> [128,4] via a tiny internal-DRAM round trip.
All matmuls fp16 (full PE speed; ~3e-4 rel err measured on HW for K=1024).
"""
import sys
import os

sys.path.insert(0, "/opt/trn_rl_repo")
from contextlib import ExitStack

import numpy as np

import concourse.bass as bass
import concourse.tile as tile
from concourse import bacc, mybir
from concourse.bass import ts

F32 = mybir.dt.float32
FP16 = mybir.dt.float16

B, TQ, TK, F, H = 8, 2048, 2048, 1024, 1024
N_CORES = 8
FA = 1152            # augmented+padded contraction dim (1024 inputs + ones + pad)
AT = FA // 128       # 9 tiles
KT = TK // 128       # 16 k-tiles
QC = TQ // 512       # 4 q-chunks
HC = H // 512        # 2 h-chunks
MASK_ADD = -3200.0   # additive mask pre-scaled by sqrt(H)=32; exp((S-3200)/32)->0
SCALE = 1.0 / 32.0


def build_nc(iters: int = 1):
    nc = bacc.Bacc("TRN2", target_bir_lowering=False, debug=False,
                   num_devices=N_CORES)
    xq_d = nc.dram_tensor("xq_t", [F, TQ], FP16, kind="ExternalInput").ap()
    xe_d = nc.dram_tensor("xe_t", [FA, TK], FP16, kind="ExternalInput").ap()
    mt_d = nc.dram_tensor("mt", [FA, FA], FP16, kind="ExternalInput").ap()
    wv_d = nc.dram_tensor("wv_aug", [FA, H], FP16, kind="ExternalInput").ap()
    mt1024_d = nc.dram_tensor("mt1024_t", [128, AT], F32, kind="ExternalInput").ap()
    mk_d = nc.dram_tensor("maskt", [TK, TQ], FP16, kind="ExternalInput").ap()
    o_d = nc.dram_tensor("o", [TQ, H], F32, kind="ExternalOutput").ap()

    with tile.TileContext(nc) as tc, ExitStack() as ctx:
        glob = ctx.enter_context(tc.tile_pool(name="glob", bufs=1))
        psum = ctx.enter_context(tc.tile_pool(name="psum", bufs=3, space="PSUM"))
        dpsum = ctx.enter_context(tc.tile_pool(name="dpsum", bufs=1, space="PSUM"))
        opsum = ctx.enter_context(tc.tile_pool(name="opsum", bufs=2, space="PSUM"))

        # ---- persistent tensors ----
        kt_sb = glob.tile([128, 8 * TK], FP16)       # 32KB/part  Ktilde^T [a][k]
        krow_sb = glob.tile([1, TK], F32)            # bq-row of Ktilde^T (/32)
        kbias = glob.tile([128, KT], F32)            # same, relaid [k-part, k-tile]
        v_sb = glob.tile([128, KT * H], FP16)        # 32KB/part  V [k][h]
        ones_col = glob.tile([128, 1], FP16)
        nc.vector.memset(ones_col[:], 1.0)
        mt1024_sb = glob.tile([128, AT], F32)

        loop_cm = tc.For_i(0, iters, 1) if iters > 1 else None
        if loop_cm is not None:
            loop_cm.__enter__()

        with ExitStack() as actx:
            pha = actx.enter_context(tc.tile_pool(name="pha", bufs=1))
            xe_sb = pha.tile([128, AT * TK], FP16)   # 36KB/part E_aug^T
            mt_sb = pha.tile([128, AT * FA], FP16)   # 20.25KB/part MT [b][a]
            wv_sb = pha.tile([128, AT * H], FP16)    # 18KB/part Wv_aug
            nc.sync.dma_start(out=mt1024_sb[:], in_=mt1024_d)
            xe_rs = xe_sb[:].rearrange("p (t j) -> p t j", t=AT)
            xe_rd = xe_d.rearrange("(t p) j -> p t j", p=128)
            mt_rs = mt_sb[:].rearrange("p (t j) -> p t j", t=AT)
            mt_rd = mt_d.rearrange("(t p) j -> p t j", p=128)
            nc.sync.dma_start(out=mt_rs[:, :, 0:640], in_=mt_rd[:, :, 0:640])
            nc.sync.dma_start(out=xe_rs[:, :, 0:512], in_=xe_rd[:, :, 0:512])
            nc.sync.dma_start(out=mt_rs[:, :, 640:FA], in_=mt_rd[:, :, 640:FA])
            for kc in range(1, QC):
                nc.sync.dma_start(out=xe_rs[:, :, kc * 512:kc * 512 + 512],
                                  in_=xe_rd[:, :, kc * 512:kc * 512 + 512])
            nc.sync.dma_start(out=wv_sb[:].rearrange("p (t j) -> p t j", t=AT),
                              in_=wv_d.rearrange("(t p) j -> p t j", p=128))

            # ---- Ktilde^T[a, k] = sum_b MT[b, a-tile].T @ E_aug^T[b, k] ----
            for kc in range(QC):
                for a in range(AT):
                    ps_kt = psum.tile([128, 512], F32, name="ps_kt", tag="pp")
                    for b_ in range(8):
                        nc.tensor.matmul(
                            ps_kt[:],
                            mt_sb[:, b_ * FA + a * 128:b_ * FA + a * 128 + 128],
                            xe_sb[:, b_ * TK + kc * 512:b_ * TK + kc * 512 + 512],
                            start=(b_ == 0), stop=(b_ == 7))
                    if a < 8:
                        nc.scalar.activation(
                            kt_sb[:, a * TK + kc * 512:a * TK + kc * 512 + 512],
                            ps_kt[:], mybir.ActivationFunctionType.Identity,
                            bias=mt1024_sb[:, a:a + 1])
                    else:
                        nc.scalar.activation(
                            krow_sb[:, kc * 512:kc * 512 + 512],
                            ps_kt[0:1, :],
                            mybir.ActivationFunctionType.Identity,
                            bias=mt1024_sb[0:1, 8:9], scale=SCALE)

            # ---- V[k, h] = sum_b E_aug^T[b, k-tile].T @ Wv_aug[b, h] ----
            for k in range(KT):
                for hc in range(HC):
                    ps_v = psum.tile([128, 512], F32, name="ps_v", tag="pp")
                    for b_ in range(AT):
                        nc.tensor.matmul(
                            ps_v[:],
                            xe_sb[:, b_ * TK + k * 128:b_ * TK + k * 128 + 128],
                            wv_sb[:, b_ * H + hc * 512:b_ * H + hc * 512 + 512],
                            start=(b_ == 0), stop=(b_ == AT - 1))
                    nc.vector.tensor_copy(
                        v_sb[:, k * H + hc * 512:k * H + hc * 512 + 512],
                        ps_v[:])

        # relayout bq-row [1, TK] -> [128, KT] via internal-DRAM round trip
        krow_dr = glob.tile([1, TK], F32, name="krow_dr", space="DRAM")
        nc.sync.dma_start(out=krow_dr[:], in_=krow_sb[:])
        nc.sync.dma_start(
            out=kbias[:],
            in_=krow_dr[:].rearrange("o (s p) -> p (o s)", p=128))

        # ---- per q-chunk: S^T, masked exp, denominators, O ----
        chunk = ctx.enter_context(tc.tile_pool(name="chunk", bufs=2))
        ppool = ctx.enter_context(tc.tile_pool(name="ppool", bufs=18))
        mpool = ctx.enter_context(tc.tile_pool(name="mpool", bufs=6))
        spool = ctx.enter_context(tc.tile_pool(name="spool", bufs=2))
        rpool = ctx.enter_context(tc.tile_pool(name="rpool", bufs=3))
        dram = ctx.enter_context(tc.tile_pool(name="dram", bufs=2, space="DRAM"))

        for c in range(QC):
            xqc = chunk.tile([128, 8 * 512], FP16, name="xqc", tag="xqc")
            nc.sync.dma_start(
                out=xqc[:].rearrange("p (t j) -> p t j", t=8),
                in_=xq_d.rearrange("(t p) q -> p t q", p=128)[
                    :, :, c * 512:(c + 1) * 512])

            # S^T tiles + masked exp -> P^T
            pt_tiles = []
            for k in range(KT):
                ps_s = psum.tile([128, 512], F32, name="ps_s", tag="pp")
                for a in range(8):
                    nc.tensor.matmul(
                        ps_s[:],
                        kt_sb[:, a * TK + k * 128:a * TK + k * 128 + 128],
                        xqc[:, a * 512:(a + 1) * 512],
                        start=(a == 0), stop=(a == 7))
                mk = mpool.tile([128, 512], FP16, name="mk")
                nc.sync.dma_start(
                    out=mk[:],
                    in_=mk_d[k * 128:(k + 1) * 128, c * 512:(c + 1) * 512])
                nc.vector.tensor_add(ps_s[:], ps_s[:], mk[:])
                pt = ppool.tile([128, 512], FP16, name="pt")
                nc.scalar.activation(pt[:], ps_s[:],
                                     mybir.ActivationFunctionType.Exp,
                                     scale=SCALE, bias=kbias[:, k:k + 1])
                pt_tiles.append(pt)

            # denominators: den[1, q] = sum_k ones.T @ P^T ; relayout to [128,4]
            ps_den = dpsum.tile([1, 512], F32, name="ps_den", tag="dpp")
            for k in range(KT):
                nc.tensor.matmul(ps_den[:], ones_col[:], pt_tiles[k][:],
                                 start=(k == 0), stop=(k == KT - 1))
            den_sb = rpool.tile([1, 512], F32, name="den_sb", tag="den_sb")
            nc.vector.tensor_copy(den_sb[:], ps_den[:])
            den_dr = dram.tile([1, 512], F32, name="den_dr")
            nc.sync.dma_start(out=den_dr[:], in_=den_sb[:])
            den_t = rpool.tile([128, 4], F32, name="den_t", tag="den_t")
            nc.sync.dma_start(
                out=den_t[:],
                in_=den_dr[:].rearrange("o (s p) -> p (o s)", p=128))
            recip = rpool.tile([128, 4], F32, name="recip", tag="recip")
            nc.vector.reciprocal(recip[:], den_t[:])

            # O[q, h] += P^T[k, qsub].T @ V
            for qs in range(4):
                ps_o = opsum.tile([128, 1024], F32, name="ps_o", tag="po")
                for k in range(KT):
                    lhs = pt_tiles[k][:, qs * 128:(qs + 1) * 128]
                    nc.tensor.matmul(ps_o[:, 0:512], lhs,
                                     v_sb[:, k * H:k * H + 512],
                                     start=(k == 0), stop=(k == KT - 1))
                    nc.tensor.matmul(ps_o[:, 512:1024], lhs,
                                     v_sb[:, k * H + 512:k * H + 1024],
                                     start=(k == 0), stop=(k == KT - 1))
                stage = spool.tile([128, 1024], F32, name="stage")
                nc.vector.tensor_scalar_mul(stage[:], ps_o[:],
                                            recip[:, qs:qs + 1])
                nc.sync.dma_start(
                    out=o_d[c * 512 + qs * 128:c * 512 + (qs + 1) * 128, :],
                    in_=stage[:])

        if loop_cm is not None:
            loop_cm.__exit__(None, None, None)

    nc.compile()
    return nc


# ---------------------------------------------------------------------------
# PJRT execution (axon) — self-contained runner
# ---------------------------------------------------------------------------
class SpmdRunner:
    def __init__(self, nc, n_cores=N_CORES):
        import jax
        from jax.sharding import Mesh, PartitionSpec
        from jax.experimental.shard_map import shard_map
        from concourse.bass2jax import (_bass_exec_p, install_neuronx_cc_hook,
                                        partition_id_tensor)

        install_neuronx_cc_hook()
        self.jax = jax
        self.nc = nc
        self.n_cores = n_cores
        in_names, out_names, out_avals, zero_outs = [], [], [], []
        for alloc in nc.m.functions[0].allocations:
            if not isinstance(alloc, mybir.MemoryLocationSet):
                continue
            name = alloc.memorylocations[0].name
            if alloc.kind == "ExternalInput":
                if (nc.partition_id_tensor is None
                        or name != nc.partition_id_tensor.name):
                    in_names.append(name)
            elif alloc.kind == "ExternalOutput":
                out_names.append(name)
                shape = tuple(alloc.tensor_shape)
                dtype = mybir.dt.np(alloc.dtype)
                out_avals.append(jax.core.ShapedArray(shape, dtype))
                zero_outs.append(np.zeros(shape, dtype))
        self.in_names, self.out_names = in_names, out_names
        self.out_avals, self.zero_outs = out_avals, zero_outs
        n_params = len(in_names)
        pname = nc.partition_id_tensor.name if nc.partition_id_tensor else None
        all_in = list(in_names) + list(out_names)
        if pname is not None:
            all_in.append(pname)

        def _body(*args):
            operands = list(args)
            if pname is not None:
                operands.append(partition_id_tensor())
            outs = _bass_exec_p.bind(
                *operands, out_avals=tuple(out_avals), in_names=tuple(all_in),
                out_names=tuple(out_names), lowering_input_output_aliases=(),
                sim_require_finite=True, sim_require_nnan=True, nc=nc)
            return tuple(outs)

        devices = jax.devices()[:n_cores]
        self.mesh = Mesh(np.asarray(devices), ("core",))
        n_outs = len(out_names)
        self.fn = jax.jit(
            shard_map(_body, mesh=self.mesh,
                      in_specs=(PartitionSpec("core"),) * (n_params + n_outs),
                      out_specs=(PartitionSpec("core"),) * n_outs,
                      check_rep=False),
            keep_unused=True)
        self._staged = None

    def stage(self, in_maps):
        from jax.sharding import NamedSharding, PartitionSpec
        n = self.n_cores
        concat = [np.concatenate([np.asarray(in_maps[c][name])
                                  for c in range(n)], axis=0)
                  for name in self.in_names]
        concat += [np.zeros((n * z.shape[0], *z.shape[1:]), z.dtype)
                   for z in self.zero_outs]
        sh = NamedSharding(self.mesh, PartitionSpec("core"))
        self._staged = [self.jax.device_put(x, sh) for x in concat]

    def run(self):
        out = self.fn(*self._staged)
        self.jax.block_until_ready(out)
        return out

    def fetch(self, out):
        res = []
        for c in range(self.n_cores):
            d = {}
            for i, name in enumerate(self.out_names):
                arr = np.asarray(out[i])
                d[name] = arr.reshape(self.n_cores, *self.out_avals[i].shape)[c]
            res.append(d)
        return res


def prep_in_maps(query, encoder_states, target_mask, Wq, bq, Wk, bk, Wv, bv):
    Wq64 = np.asarray(Wq, np.float64)
    Wk64 = np.asarray(Wk, np.float64)
    wq_aug = np.concatenate([Wq64, np.asarray(bq, np.float64)[None, :]], axis=0)
    wk_aug = np.concatenate([Wk64, np.asarray(bk, np.float64)[None, :]], axis=0)
    m_full = wk_aug @ wq_aug.T                   # MT[b, a] = (Wk_aug @ Wq_aug^T)
    mt = np.zeros((FA, FA), np.float16)
    mt[:F, :F + 1] = m_full[:F].astype(np.float16)   # row F handled via bias
    mt1024_t = np.zeros((128, AT), np.float32)       # MT[1024, :] per a-tile col
    mt1024_t[:, :8] = m_full[F, :F].reshape(8, 128).T
    mt1024_t[0, 8] = m_full[F, F] * (1.0 / 32.0)     # bq.bk, pre-scaled
    wv_aug = np.zeros((FA, H), np.float16)
    wv_aug[:F] = np.asarray(Wv, np.float16)
    wv_aug[F] = np.asarray(bv, np.float16)

    in_maps = []
    for b in range(N_CORES):
        xq_t = np.asarray(query[b]).T.astype(np.float16)
        xe_t = np.zeros((FA, TK), np.float16)
        xe_t[:F] = np.asarray(encoder_states[b]).T.astype(np.float16)
        xe_t[F] = 1.0
        maskt = np.where(np.asarray(target_mask[b]).T, np.float16(0),
                         np.float16(MASK_ADD))
        in_maps.append({
            "xq_t": np.ascontiguousarray(xq_t),
            "xe_t": np.ascontiguousarray(xe_t),
            "mt": mt, "wv_aug": wv_aug, "mt1024_t": mt1024_t,
            "maskt": np.ascontiguousarray(maskt),
        })
    return in_maps


_RUNNER_CACHE = {}


def get_runner(iters: int = 1):
    if iters not in _RUNNER_CACHE:
        nc = build_nc(iters)
        _RUNNER_CACHE[iters] = SpmdRunner(nc)
    return _RUNNER_CACHE[iters]


def kernel(query, encoder_states, target_mask, Wq, bq, Wk, bk, Wv, bv):
    r = get_runner(1)
    r.stage(prep_in_maps(query, encoder_states, target_mask,
                         Wq, bq, Wk, bk, Wv, bv))
    res = r.fetch(r.run())
    return np.stack([res[b]["o"] for b in range(N_CORES)]).astype(np.float32)


if __name__ == "__main__":
    # quick CoreSim check on one core
    from concourse.bass_interp import CoreSim
    sys.path.insert(0, os.path.dirname(os.path.abspath(__file__)))
    import reference

    inputs = {k: np.asarray(v) for k, v in reference.setup_inputs().items()}
    expected = np.asarray(reference.reference(**inputs))
    in_maps = prep_in_maps(**inputs)
    nc = build_nc(1)
    print("built; instructions:",
          sum(len(b.instructions) for fn in nc.m.functions
              for b in fn.blocks))
    sim = CoreSim(nc, trace=False)
    for name, arr in in_maps[0].items():
        sim.tensor(name)[:] = arr
    import time
    t0 = time.time()
    sim.simulate(check_with_hw=False)
    print(f"sim: {time.time() - t0:.1f}s")
    got = np.array(sim.tensor("o"))
    exp0 = expected[0]
    err = np.abs(got - exp0)
    denom = np.abs(exp0).max()
    print(f"core0 max_abs_err={err.max():.4e} rel_to_absmax={err.max() / denom:.4e}")


# revision 9
# speedup vs baseline: 1.0594x; 1.0594x over previous
"""Trainium2 Bass kernel for nn_Attention: cross-attention with projections.

Per-core (batch-sharded, 1 of B=8 rows per NeuronCore):
  K = E @ Wk + bk ; V = E @ Wv + bv ; Q = X @ Wq + bq
  S = Q K^T / sqrt(H) ; P = exp(S + mask_add)   (no max-subtraction: S ~ N(0,1))
  O = P @ V / rowsum(P)        (bv inside V is exactly equivalent to adding
                                bv after normalization since weights sum to 1)

Key algebraic folds (all host-side transforms act only on weights/layout):
  * Q K^T = X_aug (Wq_aug Wk_aug^T) E_aug^T, with X_aug=[X|1], so the host
    precomputes MT = Wk_aug @ Wq_aug^T once and the Q projection disappears:
    on-chip Ktilde^T = MT.T-contraction with E_aug^T, then S^T = Ktilde^T
    tiles contracted with X_aug^T. Biases bq/bk ride along exactly.
  * bv is folded into Wv_aug's 1025th row (against the host-appended ones row
    of E_aug), exact because softmax weights sum to one after normalization.
  * All augmented dims are zero-padded to 1152 = 9*128 for uniform tiles.
  * Everything transposed host-side => no on-chip transposes at all.
  * S^T layout [k, q]: softmax denominator = ones-row matmuls over P^T tiles,
    relaid out [1,512] -> [128,4] via a tiny internal-DRAM round trip.
All matmuls fp16 (full PE speed; ~3e-4 rel err measured on HW for K=1024).
"""
import sys
import os

sys.path.insert(0, "/opt/trn_rl_repo")
from contextlib import ExitStack

import numpy as np

import concourse.bass as bass
import concourse.tile as tile
from concourse import bacc, mybir
from concourse.bass import ts

F32 = mybir.dt.float32
FP16 = mybir.dt.float16

B, TQ, TK, F, H = 8, 2048, 2048, 1024, 1024
N_CORES = 8
FA = 1152            # augmented+padded contraction dim (1024 inputs + ones + pad)
AT = FA // 128       # 9 tiles
KT = TK // 128       # 16 k-tiles
QC = TQ // 512       # 4 q-chunks
HC = H // 512        # 2 h-chunks
MASK_ADD = -3200.0   # additive mask pre-scaled by sqrt(H)=32; exp((S-3200)/32)->0
SCALE = 1.0 / 32.0


def build_nc(iters: int = 1):
    nc = bacc.Bacc("TRN2", target_bir_lowering=False, debug=False,
                   num_devices=N_CORES)
    xq_d = nc.dram_tensor("xq_t", [F, TQ], FP16, kind="ExternalInput").ap()
    xe_d = nc.dram_tensor("xe_t", [FA, TK], FP16, kind="ExternalInput").ap()
    mt_d = nc.dram_tensor("mt", [FA, FA], FP16, kind="ExternalInput").ap()
    wv_d = nc.dram_tensor("wv_aug", [FA, H], FP16, kind="ExternalInput").ap()
    mt1024_d = nc.dram_tensor("mt1024_t", [128, AT], F32, kind="ExternalInput").ap()
    bvb_d = nc.dram_tensor("bv_bcast", [128, H], FP16, kind="ExternalInput").ap()
    mk_d = nc.dram_tensor("maskt", [TK, TQ], FP16, kind="ExternalInput").ap()
    o_d = nc.dram_tensor("o", [TQ, H], F32, kind="ExternalOutput").ap()

    with tile.TileContext(nc) as tc, ExitStack() as ctx:
        glob = ctx.enter_context(tc.tile_pool(name="glob", bufs=1))
        psum = ctx.enter_context(tc.tile_pool(name="psum", bufs=2, space="PSUM"))
        dpsum = ctx.enter_context(tc.tile_pool(name="dpsum", bufs=2, space="PSUM"))
        opsum = ctx.enter_context(tc.tile_pool(name="opsum", bufs=2, space="PSUM"))

        # ---- persistent tensors ----
        kt_sb = glob.tile([128, 8 * TK], FP16)       # 32KB/part  Ktilde^T [a][k]
        krow_sb = glob.tile([1, TK], F32)            # bq-row of Ktilde^T (/32)
        kbias = glob.tile([128, KT], F32)            # same, relaid [k-part, k-tile]
        v_sb = glob.tile([128, KT * H], FP16)        # 32KB/part  V [k][h]
        ones_col = glob.tile([128, 1], FP16)
        nc.vector.memset(ones_col[:], 1.0)
        mt1024_sb = glob.tile([128, AT], F32)
        bvb_sb = glob.tile([128, H], FP16)

        loop_cm = tc.For_i(0, iters, 1) if iters > 1 else None
        if loop_cm is not None:
            loop_cm.__enter__()

        with ExitStack() as actx:
            pha = actx.enter_context(tc.tile_pool(name="pha", bufs=1))
            xe_sb = pha.tile([128, AT * TK], FP16)   # 36KB/part E_aug^T
            mt_sb = pha.tile([128, AT * FA], FP16)   # 20.25KB/part MT [b][a]
            wv_sb = pha.tile([128, AT * H], FP16)    # 18KB/part Wv_aug
            nc.sync.dma_start(out=mt1024_sb[:], in_=mt1024_d)
            nc.sync.dma_start(out=bvb_sb[:], in_=bvb_d)
            xe_rs = xe_sb[:].rearrange("p (t j) -> p t j", t=AT)
            xe_rd = xe_d.rearrange("(t p) j -> p t j", p=128)
            mt_rs = mt_sb[:].rearrange("p (t j) -> p t j", t=AT)
            mt_rd = mt_d.rearrange("(t p) j -> p t j", p=128)
            nc.sync.dma_start(out=mt_rs[:, :, 0:640], in_=mt_rd[:, :, 0:640])
            nc.sync.dma_start(out=xe_rs[:, :, 0:512], in_=xe_rd[:, :, 0:512])
            nc.sync.dma_start(out=mt_rs[:, :, 640:FA], in_=mt_rd[:, :, 640:FA])
            for kc in range(1, QC):
                nc.sync.dma_start(out=xe_rs[:, :, kc * 512:kc * 512 + 512],
                                  in_=xe_rd[:, :, kc * 512:kc * 512 + 512])
            nc.sync.dma_start(out=wv_sb[:].rearrange("p (t j) -> p t j", t=AT),
                              in_=wv_d.rearrange("(t p) j -> p t j", p=128))

            # ---- Ktilde^T[a, k] = sum_b MT[b, a-tile].T @ E_aug^T[b, k] ----
            for kc in range(QC):
                for a in range(AT):
                    ps_kt = psum.tile([128, 512], F32, name="ps_kt", tag="pp")
                    for b_ in range(8):
                        nc.tensor.matmul(
                            ps_kt[:],
                            mt_sb[:, b_ * FA + a * 128:b_ * FA + a * 128 + 128],
                            xe_sb[:, b_ * TK + kc * 512:b_ * TK + kc * 512 + 512],
                            start=(b_ == 0), stop=(b_ == 7))
                    if a < 8:
                        nc.scalar.activation(
                            kt_sb[:, a * TK + kc * 512:a * TK + kc * 512 + 512],
                            ps_kt[:], mybir.ActivationFunctionType.Identity,
                            bias=mt1024_sb[:, a:a + 1])
                    else:
                        nc.scalar.activation(
                            krow_sb[:, kc * 512:kc * 512 + 512],
                            ps_kt[0:1, :],
                            mybir.ActivationFunctionType.Identity,
                            bias=mt1024_sb[0:1, 8:9], scale=SCALE)

            # ---- V[k, h] = sum_b E_aug^T[b, k-tile].T @ Wv_aug[b, h] ----
            for k in range(KT):
                for hc in range(HC):
                    ps_v = psum.tile([128, 512], F32, name="ps_v", tag="pp")
                    for b_ in range(8):
                        nc.tensor.matmul(
                            ps_v[:],
                            xe_sb[:, b_ * TK + k * 128:b_ * TK + k * 128 + 128],
                            wv_sb[:, b_ * H + hc * 512:b_ * H + hc * 512 + 512],
                            start=(b_ == 0), stop=(b_ == 7))
                    nc.vector.tensor_add(
                        v_sb[:, k * H + hc * 512:k * H + hc * 512 + 512],
                        ps_v[:], bvb_sb[:, hc * 512:hc * 512 + 512])

        # relayout bq-row [1, TK] -> [128, KT] via internal-DRAM round trip
        krow_dr = glob.tile([1, TK], F32, name="krow_dr", space="DRAM")
        nc.sync.dma_start(out=krow_dr[:], in_=krow_sb[:])
        nc.sync.dma_start(
            out=kbias[:],
            in_=krow_dr[:].rearrange("o (s p) -> p (o s)", p=128))

        # ---- per q-chunk: S^T, masked exp, denominators, O ----
        chunk = ctx.enter_context(tc.tile_pool(name="chunk", bufs=2))
        ppool = ctx.enter_context(tc.tile_pool(name="ppool", bufs=18))
        mpool = ctx.enter_context(tc.tile_pool(name="mpool", bufs=6))
        spool = ctx.enter_context(tc.tile_pool(name="spool", bufs=2))
        rpool = ctx.enter_context(tc.tile_pool(name="rpool", bufs=3))
        dram = ctx.enter_context(tc.tile_pool(name="dram", bufs=2, space="DRAM"))

        for c in range(QC):
            xqc = chunk.tile([128, 8 * 512], FP16, name="xqc", tag="xqc")
            nc.sync.dma_start(
                out=xqc[:].rearrange("p (t j) -> p t j", t=8),
                in_=xq_d.rearrange("(t p) q -> p t q", p=128)[
                    :, :, c * 512:(c + 1) * 512])

            # S^T tiles + masked exp -> P^T
            pt_tiles = []
            for k in range(KT):
                ps_s = psum.tile([128, 512], F32, name="ps_s", tag="pp")
                for a in range(8):
                    nc.tensor.matmul(
                        ps_s[:],
                        kt_sb[:, a * TK + k * 128:a * TK + k * 128 + 128],
                        xqc[:, a * 512:(a + 1) * 512],
                        start=(a == 0), stop=(a == 7))
                mk = mpool.tile([128, 512], FP16, name="mk")
                nc.sync.dma_start(
                    out=mk[:],
                    in_=mk_d[k * 128:(k + 1) * 128, c * 512:(c + 1) * 512])
                nc.vector.tensor_add(ps_s[:], ps_s[:], mk[:])
                pt = ppool.tile([128, 512], FP16, name="pt")
                nc.scalar.activation(pt[:], ps_s[:],
                                     mybir.ActivationFunctionType.Exp,
                                     scale=SCALE, bias=kbias[:, k:k + 1])
                pt_tiles.append(pt)

            # denominators: den[1, q] = sum_k ones.T @ P^T ; relayout to [128,4]
            ps_den = dpsum.tile([1, 512], F32, name="ps_den", tag="dpp")
            for k in range(KT):
                nc.tensor.matmul(ps_den[:], ones_col[:], pt_tiles[k][:],
                                 start=(k == 0), stop=(k == KT - 1))
            den_sb = rpool.tile([1, 512], F32, name="den_sb", tag="den_sb")
            nc.vector.tensor_copy(den_sb[:], ps_den[:])
            den_dr = dram.tile([1, 512], F32, name="den_dr")
            nc.sync.dma_start(out=den_dr[:], in_=den_sb[:])
            den_t = rpool.tile([128, 4], F32, name="den_t", tag="den_t")
            nc.sync.dma_start(
                out=den_t[:],
                in_=den_dr[:].rearrange("o (s p) -> p (o s)", p=128))
            recip = rpool.tile([128, 4], F32, name="recip", tag="recip")
            nc.vector.reciprocal(recip[:], den_t[:])

            # O[q, h] += P^T[k, qsub].T @ V
            for qs in range(4):
                ps_o = opsum.tile([128, 1024], F32, name="ps_o", tag="po")
                for k in range(KT):
                    lhs = pt_tiles[k][:, qs * 128:(qs + 1) * 128]
                    nc.tensor.matmul(ps_o[:, 0:512], lhs,
                                     v_sb[:, k * H:k * H + 512],
                                     start=(k == 0), stop=(k == KT - 1))
                    nc.tensor.matmul(ps_o[:, 512:1024], lhs,
                                     v_sb[:, k * H + 512:k * H + 1024],
                                     start=(k == 0), stop=(k == KT - 1))
                stage = spool.tile([128, 1024], F32, name="stage")
                nc.vector.tensor_scalar_mul(stage[:], ps_o[:],
                                            recip[:, qs:qs + 1])
                nc.sync.dma_start(
                    out=o_d[c * 512 + qs * 128:c * 512 + (qs + 1) * 128, :],
                    in_=stage[:])

        if loop_cm is not None:
            loop_cm.__exit__(None, None, None)

    nc.compile()
    return nc


# ---------------------------------------------------------------------------
# PJRT execution (axon) — self-contained runner
# ---------------------------------------------------------------------------
class SpmdRunner:
    def __init__(self, nc, n_cores=N_CORES):
        import jax
        from jax.sharding import Mesh, PartitionSpec
        from jax.experimental.shard_map import shard_map
        from concourse.bass2jax import (_bass_exec_p, install_neuronx_cc_hook,
                                        partition_id_tensor)

        install_neuronx_cc_hook()
        self.jax = jax
        self.nc = nc
        self.n_cores = n_cores
        in_names, out_names, out_avals, zero_outs = [], [], [], []
        for alloc in nc.m.functions[0].allocations:
            if not isinstance(alloc, mybir.MemoryLocationSet):
                continue
            name = alloc.memorylocations[0].name
            if alloc.kind == "ExternalInput":
                if (nc.partition_id_tensor is None
                        or name != nc.partition_id_tensor.name):
                    in_names.append(name)
            elif alloc.kind == "ExternalOutput":
                out_names.append(name)
                shape = tuple(alloc.tensor_shape)
                dtype = mybir.dt.np(alloc.dtype)
                out_avals.append(jax.core.ShapedArray(shape, dtype))
                zero_outs.append(np.zeros(shape, dtype))
        self.in_names, self.out_names = in_names, out_names
        self.out_avals, self.zero_outs = out_avals, zero_outs
        n_params = len(in_names)
        pname = nc.partition_id_tensor.name if nc.partition_id_tensor else None
        all_in = list(in_names) + list(out_names)
        if pname is not None:
            all_in.append(pname)

        def _body(*args):
            operands = list(args)
            if pname is not None:
                operands.append(partition_id_tensor())
            outs = _bass_exec_p.bind(
                *operands, out_avals=tuple(out_avals), in_names=tuple(all_in),
                out_names=tuple(out_names), lowering_input_output_aliases=(),
                sim_require_finite=True, sim_require_nnan=True, nc=nc)
            return tuple(outs)

        devices = jax.devices()[:n_cores]
        self.mesh = Mesh(np.asarray(devices), ("core",))
        n_outs = len(out_names)
        self.fn = jax.jit(
            shard_map(_body, mesh=self.mesh,
                      in_specs=(PartitionSpec("core"),) * (n_params + n_outs),
                      out_specs=(PartitionSpec("core"),) * n_outs,
                      check_rep=False),
            keep_unused=True)
        self._staged = None

    def stage(self, in_maps):
        from jax.sharding import NamedSharding, PartitionSpec
        n = self.n_cores
        concat = [np.concatenate([np.asarray(in_maps[c][name])
                                  for c in range(n)], axis=0)
                  for name in self.in_names]
        concat += [np.zeros((n * z.shape[0], *z.shape[1:]), z.dtype)
                   for z in self.zero_outs]
        sh = NamedSharding(self.mesh, PartitionSpec("core"))
        self._staged = [self.jax.device_put(x, sh) for x in concat]

    def run(self):
        out = self.fn(*self._staged)
        self.jax.block_until_ready(out)
        return out

    def fetch(self, out):
        res = []
        for c in range(self.n_cores):
            d = {}
            for i, name in enumerate(self.out_names):
                arr = np.asarray(out[i])
                d[name] = arr.reshape(self.n_cores, *self.out_avals[i].shape)[c]
            res.append(d)
        return res


def prep_in_maps(query, encoder_states, target_mask, Wq, bq, Wk, bk, Wv, bv):
    Wq64 = np.asarray(Wq, np.float64)
    Wk64 = np.asarray(Wk, np.float64)
    wq_aug = np.concatenate([Wq64, np.asarray(bq, np.float64)[None, :]], axis=0)
    wk_aug = np.concatenate([Wk64, np.asarray(bk, np.float64)[None, :]], axis=0)
    m_full = wk_aug @ wq_aug.T                   # MT[b, a] = (Wk_aug @ Wq_aug^T)
    mt = np.zeros((FA, FA), np.float16)
    mt[:F, :F + 1] = m_full[:F].astype(np.float16)   # row F handled via bias
    mt1024_t = np.zeros((128, AT), np.float32)       # MT[1024, :] per a-tile col
    mt1024_t[:, :8] = m_full[F, :F].reshape(8, 128).T
    mt1024_t[0, 8] = m_full[F, F] * (1.0 / 32.0)     # bq.bk, pre-scaled
    wv_aug = np.zeros((FA, H), np.float16)
    wv_aug[:F] = np.asarray(Wv, np.float16)
    bv_bcast = np.ascontiguousarray(
        np.broadcast_to(np.asarray(bv, np.float16)[None, :], (128, H)))

    in_maps = []
    for b in range(N_CORES):
        xq_t = np.asarray(query[b]).T.astype(np.float16)
        xe_t = np.zeros((FA, TK), np.float16)
        xe_t[:F] = np.asarray(encoder_states[b]).T.astype(np.float16)
        xe_t[F] = 1.0
        maskt = np.where(np.asarray(target_mask[b]).T, np.float16(0),
                         np.float16(MASK_ADD))
        in_maps.append({
            "xq_t": np.ascontiguousarray(xq_t),
            "xe_t": np.ascontiguousarray(xe_t),
            "mt": mt, "wv_aug": wv_aug, "mt1024_t": mt1024_t,
            "bv_bcast": bv_bcast,
            "maskt": np.ascontiguousarray(maskt),
        })
    return in_maps


_RUNNER_CACHE = {}


def get_runner(iters: int = 1):
    if iters not in _RUNNER_CACHE:
        nc = build_nc(iters)
        _RUNNER_CACHE[iters] = SpmdRunner(nc)
    return _RUNNER_CACHE[iters]


def kernel(query, encoder_states, target_mask, Wq, bq, Wk, bk, Wv, bv):
    r = get_runner(1)
    r.stage(prep_in_maps(query, encoder_states, target_mask,
                         Wq, bq, Wk, bk, Wv, bv))
    res = r.fetch(r.run())
    return np.stack([res[b]["o"] for b in range(N_CORES)]).astype(np.float32)


if __name__ == "__main__":
    # quick CoreSim check on one core
    from concourse.bass_interp import CoreSim
    sys.path.insert(0, os.path.dirname(os.path.abspath(__file__)))
    import reference

    inputs = {k: np.asarray(v) for k, v in reference.setup_inputs().items()}
    expected = np.asarray(reference.reference(**inputs))
    in_maps = prep_in_maps(**inputs)
    nc = build_nc(1)
    print("built; instructions:",
          sum(len(b.instructions) for fn in nc.m.functions
              for b in fn.blocks))
    sim = CoreSim(nc, trace=False)
    for name, arr in in_maps[0].items():
        sim.tensor(name)[:] = arr
    import time
    t0 = time.time()
    sim.simulate(check_with_hw=False)
    print(f"sim: {time.time() - t0:.1f}s")
    got = np.array(sim.tensor("o"))
    exp0 = expected[0]
    err = np.abs(got - exp0)
    denom = np.abs(exp0).max()
    print(f"core0 max_abs_err={err.max():.4e} rel_to_absmax={err.max() / denom:.4e}")
